# revision 8
# baseline (speedup 1.0000x reference)
"""Multi-head attention on 8 TRN2 NeuronCores.

Problem: queries [B,N,L,H,E], keys [B,N,S,H,E], values [B,N,S,H,D]
         out[b,n,l,h,:] = softmax(Q[b,n,l,h,:] @ K[b,n,:,h,:]^T / sqrt(E)) @ V[b,n,:,h,:]
with B,N,L,S,H,E,D = 4,7,512,512,8,64,64.

Sharding: head-parallel — core c computes all B*N=28 (b,n) slices for head h=c.

Per-core device kernel, per slice i (L=S=512, E=D=64, P=128):
  1. scoresT[sc] [128s, 512l] = K_sc^T(stationary) x Q^T(moving)   (PE, f32r full rate)
  2. attnT[sc]  = exp(0.125 * scoresT[sc])                          (ScalarE; no max-subtract:
     |scores|/8 <= ~6 so exp is comfortably inside f32 range)
  3. po [65, 512l] += VA_sc(stationary) x attnT[sc](moving)        (PE; VA = [V | ones] so
     row 64 of po accumulates the softmax denominator)
  4. po -> SBUF (VectorE copy), PE-transpose 128-col chunks -> [128l, 65],
     r = 1/denom (VectorE reciprocal), out = num * r (tensor_scalar), DMA out.

Host side: pre-transpose Q,K to [E, L] per slice, augment V with a ones column,
pack slice-pairs so SBUF tiles are full 128 partitions.
"""

import numpy as np

B, N, L, S, H, E, D = 4, 7, 512, 512, 8, 64, 64
NS = B * N          # 28 (b,n) slices per core
P = 128
SC = S // P         # 4 s-chunks
LC = L // P         # 4 l-chunks
DA = D + 1          # 65: V augmented with a ones column
SCALE = 1.0 / float(np.sqrt(E))

_CACHE = {}


def _build_program():
    import concourse.bass as bass
    import concourse.mybir as mybir
    import concourse.tile as tile
    from concourse import bacc
    from concourse.masks import make_identity

    f32 = mybir.dt.float32
    f32r = mybir.dt.float32r
    Exp = mybir.ActivationFunctionType.Exp

    nc = bacc.Bacc("TRN2", target_bir_lowering=False, debug=False)
    # f32r inputs: host pre-rounds to the fp32r bit pattern (fp32 with low 12
    # mantissa bits RNE-rounded away) so matmuls run at full PE rate.
    qt = nc.dram_tensor("qt", [NS // 2, P, L], f32r, kind="ExternalInput").ap()
    kt = nc.dram_tensor("kt", [NS // 2, P, S], f32r, kind="ExternalInput").ap()
    va = nc.dram_tensor("va", [NS, P, SC * DA], f32r, kind="ExternalInput").ap()
    o = nc.dram_tensor("o", [NS, L, D], f32, kind="ExternalOutput").ap()

    with tile.TileContext(nc) as tc:
        with (
            tc.tile_pool(name="const", bufs=1) as const_pool,
            tc.tile_pool(name="qk", bufs=2) as q_pool,
            tc.tile_pool(name="kk", bufs=2) as k_pool,
            tc.tile_pool(name="va", bufs=3) as va_pool,
            tc.tile_pool(name="attn", bufs=5) as at_pool,
            tc.tile_pool(name="osb", bufs=2) as osb_pool,
            tc.tile_pool(name="out", bufs=4) as ot_pool,
            tc.tile_pool(name="rec", bufs=4) as r_pool,
            tc.tile_pool(name="ps", bufs=4, space=bass.MemorySpace.PSUM) as ps_pool,
            tc.tile_pool(name="po", bufs=2, space=bass.MemorySpace.PSUM) as po_pool,
            tc.tile_pool(name="pt", bufs=2, space=bass.MemorySpace.PSUM) as pt_pool,
        ):
            ident = const_pool.tile([P, P], f32)
            make_identity(nc, ident[:])

            def epilogue(osb, i):
                # osb [65, 512]: rows 0..63 = out^T numerator, row 64 = denom
                for lc in range(LC):
                    pt = pt_pool.tile([P, DA], f32)
                    nc.tensor.transpose(
                        pt[:], osb[:, lc * P:(lc + 1) * P], ident[:DA, :DA]
                    )
                    r = r_pool.tile([P, 1], f32)
                    nc.vector.reciprocal(r[:], pt[:, D:DA])
                    ot = ot_pool.tile([P, D], f32)
                    nc.vector.tensor_scalar_mul(ot[:], pt[:, 0:D], r[:])
                    nc.sync.dma_start(o[i, lc * P:(lc + 1) * P, :], ot[:])

            prev = None
            for pair in range(NS // 2):
                qt_t = q_pool.tile([P, L], f32r)
                kt_t = k_pool.tile([P, S], f32r)
                nc.sync.dma_start(qt_t[:], qt[pair])
                nc.sync.dma_start(kt_t[:], kt[pair])
                for j in range(2):
                    i = 2 * pair + j
                    va_t = va_pool.tile([P, SC * DA], f32r)
                    nc.sync.dma_start(va_t[:], va[i])

                    rq = qt_t[j * E:(j + 1) * E, :]
                    pss = []
                    for sc in range(SC):
                        ps = ps_pool.tile([P, L], f32)
                        nc.tensor.matmul(
                            ps[:],
                            lhsT=kt_t[j * E:(j + 1) * E, sc * P:(sc + 1) * P],
                            rhs=rq,
                            start=True,
                            stop=True,
                        )
                        pss.append(ps)

                    # epilogue of the previous slice goes here on PE so the
                    # PV matmuls below have their exp() inputs ready by then
                    if prev is not None:
                        epilogue(*prev)

                    po = po_pool.tile([DA, L], f32)
                    for sc in range(SC):
                        at = at_pool.tile([P, L], f32r)
                        nc.scalar.activation(at[:], pss[sc][:], Exp, scale=SCALE)
                        nc.tensor.matmul(
                            po[:],
                            lhsT=va_t[:, sc * DA:(sc + 1) * DA],
                            rhs=at[:],
                            start=(sc == 0),
                            stop=(sc == SC - 1),
                        )
                    osb = osb_pool.tile([DA, L], f32)
                    nc.vector.tensor_copy(osb[:], po[:])
                    prev = (osb, i)
            epilogue(*prev)
    nc.compile()
    return nc


def _round_f32r(a):
    """Round fp32 -> fp32r bit pattern: RNE to 11 mantissa bits, low 12 bits
    zero. (fp32r layout == fp32 with the bottom 12 mantissa bits dropped.)"""
    u = np.ascontiguousarray(a, dtype=np.float32).view(np.uint32)
    u = u + np.uint32(0x7FF) + ((u >> np.uint32(12)) & np.uint32(1))
    u &= np.uint32(0xFFFFF000)
    return u.view(np.float32)


def _prep_inputs(queries, keys, values):
    """Per-core input dicts. Core c gets head h=c."""
    q = np.asarray(queries, dtype=np.float32)
    k = np.asarray(keys, dtype=np.float32)
    v = np.asarray(values, dtype=np.float32)

    # qt: [H, NS//2, 128, L] — Q^T per slice ([E, L]), slice-pairs packed on partitions
    qt = np.ascontiguousarray(q.transpose(3, 0, 1, 4, 2)).reshape(H, NS // 2, P, L)
    kt = np.ascontiguousarray(k.transpose(3, 0, 1, 4, 2)).reshape(H, NS // 2, P, S)

    # va: [H, NS, S, DA] with ones column, then [s -> (sc p)] packed to [NS, P, SC*DA]
    va = np.empty((H, NS, S, DA), dtype=np.float32)
    va[..., :D] = v.transpose(3, 0, 1, 2, 4).reshape(H, NS, S, D)
    va[..., D] = 1.0
    va = np.ascontiguousarray(
        va.reshape(H, NS, SC, P, DA).transpose(0, 1, 3, 2, 4)
    ).reshape(H, NS, P, SC * DA)

    qt = _round_f32r(qt)
    kt = _round_f32r(kt)
    va = _round_f32r(va)

    return [{"qt": qt[c], "kt": kt[c], "va": va[c]} for c in range(H)]


def _run(in_maps, trace=False, tmpdir=None):
    from concourse.bass_utils import run_bass_kernel_spmd

    if "nc" not in _CACHE:
        _CACHE["nc"] = _build_program()
    kwargs = {}
    if tmpdir is not None:
        kwargs["tmpdir"] = tmpdir
    return run_bass_kernel_spmd(
        _CACHE["nc"], in_maps, core_ids=list(range(H)), trace=trace, **kwargs
    )


def kernel(queries, keys, values, _trace=False, _results_out=None, _tmpdir=None):
    in_maps = _prep_inputs(queries, keys, values)
    res = _run(in_maps, trace=_trace, tmpdir=_tmpdir)
    if _results_out is not None:
        _results_out.append(res)
    # res.results[c]["o"]: [NS, L, D] for head c  ->  [B, N, L, H, D]
    out = np.stack([res.results[c]["o"] for c in range(H)], axis=0)
    out = out.reshape(H, B, N, L, D).transpose(1, 2, 3, 0, 4)
    return np.ascontiguousarray(out)


# revision 9
# speedup vs baseline: 1.0708x; 1.0708x over previous
"""Multi-head attention on 8 TRN2 NeuronCores.

Problem: queries [B,N,L,H,E], keys [B,N,S,H,E], values [B,N,S,H,D]
         out[b,n,l,h,:] = softmax(Q[b,n,l,h,:] @ K[b,n,:,h,:]^T / sqrt(E)) @ V[b,n,:,h,:]
with B,N,L,S,H,E,D = 4,7,512,512,8,64,64.

Sharding: head-parallel — core c computes all B*N=28 (b,n) slices for head h=c.

Device kernel per slice (L=S=512, E=D=64, P=128), all matmul operands fp16
(RNE-cast on host; scores/output accumulate in fp32 PSUM):
  1. scoresT [128s, 512l] chunks = K_sc^T (stationary) x Q^T (moving), two
     chunks per PSUM tile [128, 1024].
  2. attnT = exp(scores * 1/8) on ScalarE, one ACTIVATE per [128, 1024] pair
     (no max-subtraction: |scores|/8 <= ~6, exp fits fp16/fp32 comfortably).
  3. po [128, 512] += VA_sc (stationary) x attnT_sc (moving) where
     VA = [V | ones | 0-pad] so row 64 of po is the softmax denominator.
  4. rrow = 1/po[64] (VectorE), broadcast across partitions (GpSimd),
     osb = po[0:64] * rbc (VectorE), DMA out as [64, 512] (d-major; host
     transposes back to [l, d] while unsharding).

Software-pipelined one slice deep so the PE never waits on ScalarE's exp.
"""

import numpy as np

B, N, L, S, H, E, D = 4, 7, 512, 512, 8, 64, 64
NS = B * N          # 28 (b,n) slices per core
NP = NS // 2        # 14 slice-pairs
P = 128
SC = S // P         # 4 s-chunks
SCALE = 1.0 / float(np.sqrt(E))

# input pack layout (fp16), per slice-pair: [128, 2048] =
#   [0:512)     qtT pair  (rows 0-63 = slice a's [E, L], rows 64-127 = slice b)
#   [512:1024)  ktT pair  (same row split, cols = S)
#   [1024:1536) VA slice a: 4 s-chunks x 128 cols = [V | ones | zeros]
#   [1536:2048) VA slice b
QOFF, KOFF, VOFF = 0, 512, 1024

_CACHE = {}


def _build_program():
    import concourse.mybir as mybir
    import concourse.tile as tile
    from concourse import bacc
    import concourse.bass as bass

    f32 = mybir.dt.float32
    f16 = mybir.dt.float16
    Exp = mybir.ActivationFunctionType.Exp

    nc = bacc.Bacc("TRN2", target_bir_lowering=False, debug=False)
    inp = nc.dram_tensor("inp", [NP, P, 2048], f16, kind="ExternalInput").ap()
    o = nc.dram_tensor("o", [NS, D, L], f32, kind="ExternalOutput").ap()

    with tile.TileContext(nc) as tc:
        with (
            tc.tile_pool(name="inpool", bufs=3) as in_pool,
            tc.tile_pool(name="attn", bufs=4) as at_pool,
            tc.tile_pool(name="rrow", bufs=3) as r_pool,
            tc.tile_pool(name="rbc", bufs=3) as rbc_pool,
            tc.tile_pool(name="osb", bufs=3) as osb_pool,
            tc.tile_pool(name="ps", bufs=3, space=bass.MemorySpace.PSUM) as ps_pool,
            tc.tile_pool(name="po", bufs=2, space=bass.MemorySpace.PSUM) as po_pool,
        ):
            def emit_pv_epilogue(state):
                in_t, j, ats, i = state
                po = po_pool.tile([P, L], f32)
                for sc in range(SC):
                    nc.tensor.matmul(
                        po[:],
                        lhsT=in_t[:, VOFF + j * 512 + sc * P: VOFF + j * 512 + (sc + 1) * P],
                        rhs=ats[sc // 2][:, (sc % 2) * L:(sc % 2 + 1) * L],
                        start=(sc == 0),
                        stop=(sc == SC - 1),
                    )
                rrow = r_pool.tile([1, L], f32)
                nc.vector.reciprocal(rrow[:], po[D:D + 1, :])
                rbc = rbc_pool.tile([D, L], f32)
                nc.gpsimd.partition_broadcast(rbc[:], rrow[:])
                osb = osb_pool.tile([D, L], f32)
                nc.vector.tensor_mul(osb[:], po[0:D, :], rbc[:])
                nc.sync.dma_start(o[i], osb[:])

            state = None
            for pair in range(NP):
                in_t = in_pool.tile([P, 2048], f16)
                nc.sync.dma_start(in_t[:], inp[pair])
                for j in range(2):
                    i = 2 * pair + j
                    rq = in_t[j * E:(j + 1) * E, QOFF:QOFF + L]
                    ats = []
                    for half in range(2):
                        ps = ps_pool.tile([P, 2 * L], f32)
                        for k in range(2):
                            sc = 2 * half + k
                            nc.tensor.matmul(
                                ps[:, k * L:(k + 1) * L],
                                lhsT=in_t[j * E:(j + 1) * E, KOFF + sc * P:KOFF + (sc + 1) * P],
                                rhs=rq,
                                start=True,
                                stop=True,
                            )
                        at = at_pool.tile([P, 2 * L], f16)
                        nc.scalar.activation(at[:], ps[:], Exp, scale=SCALE)
                        ats.append(at)
                    if state is not None:
                        emit_pv_epilogue(state)
                    state = (in_t, j, ats, i)
            emit_pv_epilogue(state)
    nc.compile()
    return nc


def _prep_inputs(queries, keys, values):
    """Pack per-core fp16 inputs. Core c gets head h=c."""
    q = np.asarray(queries, dtype=np.float32)
    k = np.asarray(keys, dtype=np.float32)
    v = np.asarray(values, dtype=np.float32)

    # [H, NP, 128, 512] — Q^T/K^T per slice, slice-pairs stacked on partitions
    qt = np.ascontiguousarray(q.transpose(3, 0, 1, 4, 2)).reshape(H, NP, P, L)
    kt = np.ascontiguousarray(k.transpose(3, 0, 1, 4, 2)).reshape(H, NP, P, S)

    # VA: [H, NS, SC, 128 s, 128 cols] = [V | ones | zeros] -> [H, NP, 128, 1024]
    va = np.zeros((H, NS, SC, P, P), dtype=np.float32)
    va[..., :D] = v.transpose(3, 0, 1, 2, 4).reshape(H, NS, SC, P, D)
    va[..., D] = 1.0
    va = va.transpose(0, 1, 3, 2, 4).reshape(H, NP, 2, P, SC * P)
    va = np.ascontiguousarray(va.transpose(0, 1, 3, 2, 4)).reshape(H, NP, P, 2 * SC * P)

    inp = np.concatenate([qt, kt, va], axis=-1).astype(np.float16)
    return [{"inp": inp[c]} for c in range(H)]


def _run(in_maps, trace=False, tmpdir=None):
    from concourse.bass_utils import run_bass_kernel_spmd

    if "nc" not in _CACHE:
        _CACHE["nc"] = _build_program()
    kwargs = {}
    if tmpdir is not None:
        kwargs["tmpdir"] = tmpdir
    return run_bass_kernel_spmd(
        _CACHE["nc"], in_maps, core_ids=list(range(H)), trace=trace, **kwargs
    )


def kernel(queries, keys, values, _trace=False, _results_out=None, _tmpdir=None):
    in_maps = _prep_inputs(queries, keys, values)
    res = _run(in_maps, trace=_trace, tmpdir=_tmpdir)
    if _results_out is not None:
        _results_out.append(res)
    # res.results[c]["o"]: [NS, D, L] for head c  ->  [B, N, L, H, D]
    out = np.stack([res.results[c]["o"] for c in range(H)], axis=0)
    out = out.reshape(H, B, N, D, L).transpose(1, 2, 4, 0, 3)
    return np.ascontiguousarray(out)


# revision 12
# speedup vs baseline: 1.4998x; 1.4007x over previous
"""Multi-head attention on 8 TRN2 NeuronCores.

Problem: queries [B,N,L,H,E], keys [B,N,S,H,E], values [B,N,S,H,D]
         out[b,n,l,h,:] = softmax(Q[b,n,l,h,:] @ K[b,n,:,h,:]^T / sqrt(E)) @ V[b,n,:,h,:]
with B,N,L,S,H,E,D = 4,7,512,512,8,64,64.

Sharding: head-parallel — core c computes all B*N=28 (b,n) slices for head h=c.

Device kernel per slice (L=S=512, E=D=64, P=128), all matmul operands fp16
(RNE-cast on host; scores/output accumulate in fp32 PSUM):
  1. scoresT [128s, 512l] chunks = K_sc^T (stationary) x Q^T (moving), two
     chunks per PSUM tile [128, 1024].
  2. attnT = exp(scores * 1/8) on ScalarE, one ACTIVATE per [128, 1024] pair
     (no max-subtraction: |scores|/8 <= ~6, exp fits fp16/fp32 comfortably).
  3. po [128, 512] += VA_sc (stationary) x attnT_sc (moving) where
     VA = [V | ones | 0-pad] so row 64 of po is the softmax denominator.
  4. rrow = 1/po[64] (VectorE), broadcast across partitions (GpSimd),
     osb = po[0:64] * rbc (VectorE), DMA out as [64, 512] (d-major; host
     transposes back to [l, d] while unsharding).

Software-pipelined one slice deep so the PE never waits on ScalarE's exp.
"""

import numpy as np

B, N, L, S, H, E, D = 4, 7, 512, 512, 8, 64, 64
NS = B * N          # 28 (b,n) slices per core
NP = NS // 2        # 14 slice-pairs
P = 128
SC = S // P         # 4 s-chunks
SCALE = 1.0 / float(np.sqrt(E))

# input pack layout (fp16), per slice-pair: [128, 2048] =
#   [0:512)     qtT pair  (rows 0-63 = slice a's [E, L], rows 64-127 = slice b)
#   [512:1024)  ktT pair  (same row split, cols = S)
#   [1024:1536) VA slice a: 4 s-chunks x 128 cols = [V | ones | zeros]
#   [1536:2048) VA slice b
QOFF, KOFF, VOFF = 0, 512, 1024

_CACHE = {}


def _build_program():
    import concourse.mybir as mybir
    import concourse.tile as tile
    from concourse import bacc
    import concourse.bass as bass

    f32 = mybir.dt.float32
    f16 = mybir.dt.float16
    Exp = mybir.ActivationFunctionType.Exp

    nc = bacc.Bacc("TRN2", target_bir_lowering=False, debug=False)
    inp = nc.dram_tensor("inp", [NP, P, 2048], f16, kind="ExternalInput").ap()
    o = nc.dram_tensor("o", [NS, D, L], f32, kind="ExternalOutput").ap()

    with tile.TileContext(nc) as tc:
        with (
            tc.tile_pool(name="inpool", bufs=3) as in_pool,
            tc.tile_pool(name="attn", bufs=4) as at_pool,
            tc.tile_pool(name="rrow", bufs=3) as r_pool,
            tc.tile_pool(name="rbc", bufs=3) as rbc_pool,
            tc.tile_pool(name="osb", bufs=3) as osb_pool,
            tc.tile_pool(name="ps", bufs=3, space=bass.MemorySpace.PSUM) as ps_pool,
            tc.tile_pool(name="po", bufs=2, space=bass.MemorySpace.PSUM) as po_pool,
        ):
            def emit_pv_epilogue(state):
                in_t, j, ats, i = state
                po = po_pool.tile([P, L], f32)
                for sc in range(SC):
                    nc.tensor.matmul(
                        po[:],
                        lhsT=in_t[:, VOFF + j * 512 + sc * P: VOFF + j * 512 + (sc + 1) * P],
                        rhs=ats[sc // 2][:, (sc % 2) * L:(sc % 2 + 1) * L],
                        start=(sc == 0),
                        stop=(sc == SC - 1),
                    )
                # VA = [ones | 0*63 | V]: po[0] = denom (partition 0 — the
                # custom-DVE recip mishandles nonzero partition offsets),
                # po[64:128] = numerator^T (32-aligned partition start).
                rrow = r_pool.tile([1, L], f32)
                nc.vector.reciprocal_approx_fast(rrow[:], po[0:1, :])
                rbc = rbc_pool.tile([D, L], f32)
                nc.gpsimd.partition_broadcast(rbc[:], rrow[:])
                osb = osb_pool.tile([D, L], f32)
                nc.vector.tensor_mul(osb[:], po[D:2 * D, :], rbc[:])
                nc.sync.dma_start(o[i], osb[:])

            state = None
            for pair in range(NP):
                in_t = in_pool.tile([P, 2048], f16)
                nc.sync.dma_start(in_t[:], inp[pair])
                for j in range(2):
                    i = 2 * pair + j
                    rq = in_t[j * E:(j + 1) * E, QOFF:QOFF + L]
                    ats = []
                    for half in range(2):
                        ps = ps_pool.tile([P, 2 * L], f32)
                        for k in range(2):
                            sc = 2 * half + k
                            nc.tensor.matmul(
                                ps[:, k * L:(k + 1) * L],
                                lhsT=in_t[j * E:(j + 1) * E, KOFF + sc * P:KOFF + (sc + 1) * P],
                                rhs=rq,
                                start=True,
                                stop=True,
                            )
                        at = at_pool.tile([P, 2 * L], f16)
                        nc.scalar.activation(at[:], ps[:], Exp, scale=SCALE)
                        ats.append(at)
                    if state is not None:
                        emit_pv_epilogue(state)
                    state = (in_t, j, ats, i)
            emit_pv_epilogue(state)
    nc.compile()
    return nc


def _prep_inputs(queries, keys, values):
    """Pack per-core fp16 inputs. Core c gets head h=c."""
    q = np.asarray(queries, dtype=np.float32)
    k = np.asarray(keys, dtype=np.float32)
    v = np.asarray(values, dtype=np.float32)

    # [H, NP, 128, 512] — Q^T/K^T per slice, slice-pairs stacked on partitions
    qt = np.ascontiguousarray(q.transpose(3, 0, 1, 4, 2)).reshape(H, NP, P, L)
    kt = np.ascontiguousarray(k.transpose(3, 0, 1, 4, 2)).reshape(H, NP, P, S)

    # VA: [H, NS, SC, 128 s, 128 cols] = [ones | zeros | V] -> [H, NP, 128, 1024]
    va = np.zeros((H, NS, SC, P, P), dtype=np.float32)
    va[..., D:2 * D] = v.transpose(3, 0, 1, 2, 4).reshape(H, NS, SC, P, D)
    va[..., 0] = 1.0
    va = va.transpose(0, 1, 3, 2, 4).reshape(H, NP, 2, P, SC * P)
    va = np.ascontiguousarray(va.transpose(0, 1, 3, 2, 4)).reshape(H, NP, P, 2 * SC * P)

    inp = np.concatenate([qt, kt, va], axis=-1).astype(np.float16)
    return [{"inp": inp[c]} for c in range(H)]


def _run(in_maps, trace=False, tmpdir=None):
    from concourse.bass_utils import run_bass_kernel_spmd

    if "nc" not in _CACHE:
        _CACHE["nc"] = _build_program()
    kwargs = {}
    if tmpdir is not None:
        kwargs["tmpdir"] = tmpdir
    return run_bass_kernel_spmd(
        _CACHE["nc"], in_maps, core_ids=list(range(H)), trace=trace, **kwargs
    )


def kernel(queries, keys, values, _trace=False, _results_out=None, _tmpdir=None):
    in_maps = _prep_inputs(queries, keys, values)
    res = _run(in_maps, trace=_trace, tmpdir=_tmpdir)
    if _results_out is not None:
        _results_out.append(res)
    # res.results[c]["o"]: [NS, D, L] for head c  ->  [B, N, L, H, D]
    out = np.stack([res.results[c]["o"] for c in range(H)], axis=0)
    out = out.reshape(H, B, N, D, L).transpose(1, 2, 4, 0, 3)
    return np.ascontiguousarray(out)


# revision 14
# speedup vs baseline: 1.8596x; 1.2399x over previous
"""Multi-head attention on 8 TRN2 NeuronCores.

Problem: queries [B,N,L,H,E], keys [B,N,S,H,E], values [B,N,S,H,D]
         out[b,n,l,h,:] = softmax(Q[b,n,l,h,:] @ K[b,n,:,h,:]^T / sqrt(E)) @ V[b,n,:,h,:]
with B,N,L,S,H,E,D = 4,7,512,512,8,64,64.

Sharding: head-parallel — core c computes all B*N=28 (b,n) slices for head h=c.

Device kernel per slice (L=S=512, E=D=64, P=128), all matmul operands fp16
(RNE-cast on host; scores/output accumulate in fp32 PSUM):
  1. scoresT [128s, 512l] chunks = K_sc^T (stationary) x Q^T (moving), two
     chunks per PSUM tile [128, 1024].
  2. attnT = exp(scores * 1/8) on ScalarE, one ACTIVATE per [128, 1024] pair
     (no max-subtraction: |scores|/8 <= ~6, exp fits fp16/fp32 comfortably).
  3. po [128, 512] += VA_sc (stationary) x attnT_sc (moving) where
     VA = [V | ones | 0-pad] so row 64 of po is the softmax denominator.
  4. rrow = 1/po[64] (VectorE), broadcast across partitions (GpSimd),
     osb = po[0:64] * rbc (VectorE), DMA out as [64, 512] (d-major; host
     transposes back to [l, d] while unsharding).

Software-pipelined one slice deep so the PE never waits on ScalarE's exp.
"""

import numpy as np

B, N, L, S, H, E, D = 4, 7, 512, 512, 8, 64, 64
NS = B * N          # 28 (b,n) slices per core
NP = NS // 2        # 14 slice-pairs
P = 128
SC = S // P         # 4 s-chunks
SCALE = 1.0 / float(np.sqrt(E))

# input pack layout (fp16), per slice-pair: [128, 2048] =
#   [0:512)     qtT pair  (rows 0-63 = slice a's [E, L], rows 64-127 = slice b)
#   [512:1024)  ktT pair  (same row split, cols = S)
#   [1024:1536) VA slice a: 4 s-chunks x 128 cols = [V | ones | zeros]
#   [1536:2048) VA slice b
QOFF, KOFF, VOFF = 0, 512, 1024

_CACHE = {}


def _build_program():
    import concourse.mybir as mybir
    import concourse.tile as tile
    from concourse import bacc
    import concourse.bass as bass

    f32 = mybir.dt.float32
    f16 = mybir.dt.float16
    Exp = mybir.ActivationFunctionType.Exp

    nc = bacc.Bacc("TRN2", target_bir_lowering=False, debug=False)
    inp = nc.dram_tensor("inp", [NP, P, 2048], f16, kind="ExternalInput").ap()
    o = nc.dram_tensor("o", [NS, D, L], f32, kind="ExternalOutput").ap()

    with tile.TileContext(nc) as tc:
        with (
            tc.tile_pool(name="inpool", bufs=3) as in_pool,
            tc.tile_pool(name="attn", bufs=4) as at_pool,
            tc.tile_pool(name="rrow", bufs=3) as r_pool,
            tc.tile_pool(name="rbc", bufs=3) as rbc_pool,
            tc.tile_pool(name="osb", bufs=3) as osb_pool,
            tc.tile_pool(name="ps", bufs=3, space=bass.MemorySpace.PSUM) as ps_pool,
            tc.tile_pool(name="po", bufs=2, space=bass.MemorySpace.PSUM) as po_pool,
        ):
            def emit_pv_epilogue(state):
                in_t, j, ats, i = state
                po = po_pool.tile([P, L], f32)
                for sc in range(SC):
                    nc.tensor.matmul(
                        po[:],
                        lhsT=in_t[:, VOFF + j * 512 + sc * P: VOFF + j * 512 + (sc + 1) * P],
                        rhs=ats[sc // 2][:, (sc % 2) * L:(sc % 2 + 1) * L],
                        start=(sc == 0),
                        stop=(sc == SC - 1),
                    )
                # VA = [ones | 0*63 | V]: po[0] = denom (partition 0 — the
                # custom-DVE recip mishandles nonzero partition offsets),
                # po[64:128] = numerator^T (32-aligned partition start).
                rrow = r_pool.tile([1, L], f32)
                nc.vector.reciprocal_approx_fast(rrow[:], po[0:1, :])
                rbc = rbc_pool.tile([D, L], f32)
                nc.gpsimd.partition_broadcast(rbc[:], rrow[:])
                osb = osb_pool.tile([D, L], f32)
                nc.vector.tensor_mul(osb[:], po[D:2 * D, :], rbc[:])
                nc.sync.dma_start(o[i], osb[:])

            state = None
            for pair in range(NP):
                in_t = in_pool.tile([P, 2048], f16)
                nc.sync.dma_start(in_t[:], inp[pair])
                if pair == 0:
                    # HAM warm-up: ~9 back-to-back dummy matmuls give the PE
                    # one full 3.4us continuous-busy window so the clock gate
                    # opens (1.2 -> 2.4 GHz). Without this the kernel's
                    # per-slice dependency stalls keep every busy burst under
                    # the window and the whole kernel runs at half clock.
                    wps = po_pool.tile([P, L], f32, tag="po")
                    for _ in range(9):
                        nc.tensor.matmul(
                            wps[:],
                            lhsT=in_t[:, VOFF:VOFF + P],
                            rhs=in_t[:, 0:L],
                            start=True,
                            stop=True,
                        )
                for j in range(2):
                    i = 2 * pair + j
                    rq = in_t[j * E:(j + 1) * E, QOFF:QOFF + L]
                    ats = []
                    for half in range(2):
                        ps = ps_pool.tile([P, 2 * L], f32)
                        for k in range(2):
                            sc = 2 * half + k
                            nc.tensor.matmul(
                                ps[:, k * L:(k + 1) * L],
                                lhsT=in_t[j * E:(j + 1) * E, KOFF + sc * P:KOFF + (sc + 1) * P],
                                rhs=rq,
                                start=True,
                                stop=True,
                            )
                        at = at_pool.tile([P, 2 * L], f16)
                        nc.scalar.activation(at[:], ps[:], Exp, scale=SCALE)
                        ats.append(at)
                    if state is not None:
                        emit_pv_epilogue(state)
                    state = (in_t, j, ats, i)
            emit_pv_epilogue(state)
    nc.compile()
    return nc


def _prep_inputs(queries, keys, values):
    """Pack per-core fp16 inputs. Core c gets head h=c."""
    q = np.asarray(queries, dtype=np.float32)
    k = np.asarray(keys, dtype=np.float32)
    v = np.asarray(values, dtype=np.float32)

    # [H, NP, 128, 512] — Q^T/K^T per slice, slice-pairs stacked on partitions
    qt = np.ascontiguousarray(q.transpose(3, 0, 1, 4, 2)).reshape(H, NP, P, L)
    kt = np.ascontiguousarray(k.transpose(3, 0, 1, 4, 2)).reshape(H, NP, P, S)

    # VA: [H, NS, SC, 128 s, 128 cols] = [ones | zeros | V] -> [H, NP, 128, 1024]
    va = np.zeros((H, NS, SC, P, P), dtype=np.float32)
    va[..., D:2 * D] = v.transpose(3, 0, 1, 2, 4).reshape(H, NS, SC, P, D)
    va[..., 0] = 1.0
    va = va.transpose(0, 1, 3, 2, 4).reshape(H, NP, 2, P, SC * P)
    va = np.ascontiguousarray(va.transpose(0, 1, 3, 2, 4)).reshape(H, NP, P, 2 * SC * P)

    inp = np.concatenate([qt, kt, va], axis=-1).astype(np.float16)
    return [{"inp": inp[c]} for c in range(H)]


def _run(in_maps, trace=False, tmpdir=None):
    from concourse.bass_utils import run_bass_kernel_spmd

    if "nc" not in _CACHE:
        _CACHE["nc"] = _build_program()
    kwargs = {}
    if tmpdir is not None:
        kwargs["tmpdir"] = tmpdir
    return run_bass_kernel_spmd(
        _CACHE["nc"], in_maps, core_ids=list(range(H)), trace=trace, **kwargs
    )


def kernel(queries, keys, values, _trace=False, _results_out=None, _tmpdir=None):
    in_maps = _prep_inputs(queries, keys, values)
    res = _run(in_maps, trace=_trace, tmpdir=_tmpdir)
    if _results_out is not None:
        _results_out.append(res)
    # res.results[c]["o"]: [NS, D, L] for head c  ->  [B, N, L, H, D]
    out = np.stack([res.results[c]["o"] for c in range(H)], axis=0)
    out = out.reshape(H, B, N, D, L).transpose(1, 2, 4, 0, 3)
    return np.ascontiguousarray(out)
